# revision 1
# baseline (speedup 1.0000x reference)
"""Fused pre-norm transformer block (attention + MLP) on 8 TRN2 NeuronCores.

Sharding: token-parallel, cyclic. Core c = b*4 + r owns tokens t of batch b
with t % 4 == r (512 tokens/core). Weights replicated (bf16). One fused
K/V AllGather (bf16) per batch group {0..3}, {4..7}. Causal structure is
uniform across cores: with cyclic sharding, key-slot r' vs own rank r needs
triangular mask "j<=i" (r'<=r) or "j<i" (r'>r) — passed as per-core input
tiles, so all 8 cores run one identical SPMD graph.

Layouts on device (per core):
  activations feature-major  [D, 512]  (tokens on free axis)
  LN stats via ones-matmul (partition reduction), broadcast via K=1 matmul
  scores^T [k, q] so AV matmul needs no transpose; V is augmented with a
  ones column per head (65-wide) so softmax denominators fall out of the
  AV matmul's row 64.
"""

import numpy as np

from concourse import bass, bacc, tile, mybir
from concourse.bass_utils import run_bass_kernel_spmd

F32 = mybir.dt.float32
BF16 = mybir.dt.bfloat16
U8 = mybir.dt.uint8
NPBF16 = mybir.dt.np(BF16)

D = 1024
H = 16
HD = 64
FF = 4096
B = 2
L = 2048
R = 4            # ranks per batch group
T = 512          # tokens per core
NT = T // 128    # 4 token tiles per core
DT = D // 128    # 8 feature tiles
FT = FF // 128   # 32 ff tiles
VW = HD + 1      # 65: v + ones column
VROW = H * VW    # 1040
K_ELEMS = D * T          # 524288
V_ELEMS = T * VROW       # 532480
KV_ELEMS = K_ELEMS + V_ELEMS
SCALE = HD ** -0.5
NEG = -30000.0
AX = mybir.AluOpType

_CACHE = {}


def _layernorm(tc, sbp, x3d, gam, bet, ones_col, ones_row, out3d):
    """x3d [128, DT, 512] f32 -> out3d [128, DT, 512] bf16 (feature-major)."""
    nc = tc.nc
    with tc.tile_pool(name="lnps", bufs=2, space="PSUM") as psp:
        sums = psp.tile([1, T], F32, tag="lnstat")
        ssum = psp.tile([1, T], F32, tag="lnstat")
        for a in range(DT):
            xsq = sbp.tile([128, T], F32, tag="lnsq")
            nc.vector.tensor_tensor(xsq[:], x3d[:, a, :], x3d[:, a, :], AX.mult)
            nc.tensor.matmul(sums[:], ones_col[:], x3d[:, a, :],
                             start=(a == 0), stop=(a == DT - 1))
            nc.tensor.matmul(ssum[:], ones_col[:], xsq[:],
                             start=(a == 0), stop=(a == DT - 1))
        mean = sbp.tile([1, T], F32)
        ess = sbp.tile([1, T], F32)
        var = sbp.tile([1, T], F32)
        sd = sbp.tile([1, T], F32)
        s = sbp.tile([1, T], F32)
        m = sbp.tile([1, T], F32)
        nc.vector.tensor_scalar_mul(mean[:], sums[:], 1.0 / D)
        nc.vector.tensor_scalar_mul(ess[:], ssum[:], 1.0 / D)
        nc.vector.tensor_tensor(var[:], mean[:], mean[:], AX.mult)
        nc.vector.tensor_tensor(var[:], ess[:], var[:], AX.subtract)
        nc.vector.tensor_scalar_add(var[:], var[:], 1e-5)
        nc.scalar.activation(sd[:], var[:], mybir.ActivationFunctionType.Sqrt)
        nc.vector.reciprocal(s[:], sd[:])
        nc.vector.tensor_tensor(m[:], mean[:], s[:], AX.mult)
        nc.vector.tensor_scalar_mul(m[:], m[:], -1.0)
        for a in range(DT):
            bs = psp.tile([128, T], F32, tag="lnbc", bufs=4)
            bm = psp.tile([128, T], F32, tag="lnbc", bufs=4)
            ga = gam[0:1, a * 128:(a + 1) * 128]
            nc.tensor.matmul(bs[:], ga, s[:], start=True, stop=True)
            nc.tensor.matmul(bm[:], ga, m[:], start=True, stop=False)
            nc.tensor.matmul(bm[:], bet[0:1, a * 128:(a + 1) * 128],
                             ones_row[:], start=False, stop=True)
            tmp = sbp.tile([128, T], F32, tag="lntmp")
            nc.vector.tensor_tensor(tmp[:], x3d[:, a, :], bs[:], AX.mult)
            nc.vector.tensor_tensor(out3d[:, a, :], tmp[:], bm[:], AX.add)


def _build():
    nc = bacc.Bacc("TRN2", target_bir_lowering=False, debug=False,
                   num_devices=8)
    p = {}
    p["xT"] = nc.dram_tensor("xT", [D, T], F32, kind="ExternalInput")
    p["sm"] = nc.dram_tensor("sm", [L], U8, kind="ExternalInput")
    p["cmask"] = nc.dram_tensor("cmask", [R, 128, 128], F32,
                                kind="ExternalInput")
    for w in ["wqT", "wkT", "wvT", "woT"]:
        p[w] = nc.dram_tensor(w, [D, D], BF16, kind="ExternalInput")
    p["w1T"] = nc.dram_tensor("w1T", [D, FF], BF16, kind="ExternalInput")
    p["w2T"] = nc.dram_tensor("w2T", [FF, D], BF16, kind="ExternalInput")
    for g in ["g1", "b1", "g2", "b2"]:
        p[g] = nc.dram_tensor(g, [1, D], F32, kind="ExternalInput")
    p["bm1"] = nc.dram_tensor("bm1", [FF], F32, kind="ExternalInput")
    p["bm2"] = nc.dram_tensor("bm2", [D], F32, kind="ExternalInput")
    out_h = nc.dram_tensor("out", [D, T], F32, kind="ExternalOutput")

    EXP = mybir.ActivationFunctionType.Exp
    GELU = mybir.ActivationFunctionType.Gelu

    with tile.TileContext(nc) as tc:
        with (
            tc.tile_pool(name="dram", bufs=1, space="DRAM") as dram,
            tc.tile_pool(name="persist", bufs=1) as pp,
            tc.tile_pool(name="work", bufs=2) as wk,
        ):
            kv_own = dram.tile([KV_ELEMS], BF16)
            kv_g = dram.tile([R, KV_ELEMS], BF16)

            # ---- constants / small inputs ----
            ones_col = pp.tile([128, 1], F32)
            nc.vector.memset(ones_col[:], 1.0)
            ones_row = pp.tile([1, T], F32)
            nc.vector.memset(ones_row[:], 1.0)
            ones_v = pp.tile([128, NT, H], BF16)
            nc.vector.memset(ones_v[:], 1.0)
            gb = {}
            for g in ["g1", "b1", "g2", "b2"]:
                gb[g] = pp.tile([1, D], F32, name=g + "_sb")
                nc.sync.dma_start(gb[g][:], p[g][:])
            cm = pp.tile([128, R, 128], F32)
            nc.sync.dma_start(cm[:], p["cmask"][:].rearrange("r p c -> p r c"))
            sm_u8 = pp.tile([128, R, R], U8)
            nc.sync.dma_start(
                sm_u8[:],
                p["sm"][:].rearrange("(jt p r) -> p r jt", p=128, r=R))
            seqb = pp.tile([128, R, R], F32)
            nc.vector.tensor_copy(seqb[:], sm_u8[:])
            nc.vector.tensor_scalar_mul(seqb[:], seqb[:], NEG)
            bm1c = pp.tile([128, FT], F32)
            nc.sync.dma_start(bm1c[:],
                              p["bm1"][:].rearrange("(o p) -> p o", p=128))
            bm2c = pp.tile([128, DT], F32)
            nc.sync.dma_start(bm2c[:],
                              p["bm2"][:].rearrange("(o p) -> p o", p=128))

            # ---- x, LN1 ----
            xt = pp.tile([128, DT, T], F32)
            nc.sync.dma_start(xt[:],
                              p["xT"][:].rearrange("(a p) t -> p a t", p=128))
            nx = pp.tile([128, DT, T], BF16)
            _layernorm(tc, wk, xt, gb["g1"], gb["b1"], ones_col, ones_row, nx)

            with (
                tc.tile_pool(name="wproj", bufs=2) as wpool,
                tc.tile_pool(name="kvq", bufs=1) as kvq,
            ):
                # ---- K, V projections -> kv_own ; AllGather ; Q ----
                with (
                    tc.tile_pool(name="kvtmp", bufs=1) as kvt,
                    tc.tile_pool(name="psC", bufs=3, space="PSUM") as psC,
                ):
                    wk_sb = wpool.tile([128, DT, D], BF16, tag="w")
                    nc.sync.dma_start(
                        wk_sb[:],
                        p["wkT"][:].rearrange("(a p) o -> p a o", p=128))
                    kt_sb = kvt.tile([128, DT, T], BF16, name="kt_sb")
                    for o in range(DT):
                        ps = psC.tile([128, T], F32, tag="pp")
                        for i in range(DT):
                            nc.tensor.matmul(
                                ps[:], wk_sb[:, i, o * 128:(o + 1) * 128],
                                nx[:, i, :], start=(i == 0),
                                stop=(i == DT - 1))
                        nc.vector.tensor_copy(kt_sb[:, o, :], ps[:])
                    nc.sync.dma_start(
                        kv_own[0:K_ELEMS].rearrange("(a p t) -> p a t",
                                                    p=128, t=T),
                        kt_sb[:])

                    wv_sb = wpool.tile([128, DT, D], BF16, tag="w")
                    nc.sync.dma_start(
                        wv_sb[:],
                        p["wvT"][:].rearrange("(a p) o -> p a o", p=128))
                    v_sb = kvt.tile([128, NT, D], BF16, name="v_sb")
                    for tt in range(NT):
                        for oo in range(2):
                            ps = psC.tile([128, T], F32, tag="pp")
                            for i in range(DT):
                                nc.tensor.matmul(
                                    ps[:], nx[:, i, tt * 128:(tt + 1) * 128],
                                    wv_sb[:, i, oo * 512:(oo + 1) * 512],
                                    start=(i == 0), stop=(i == DT - 1))
                            nc.vector.tensor_copy(
                                v_sb[:, tt, oo * 512:(oo + 1) * 512], ps[:])
                    v_dst = kv_own[K_ELEMS:KV_ELEMS]
                    for h in range(H):
                        nc.sync.dma_start(
                            v_dst.rearrange("(tt p c) -> p tt c", p=128,
                                            c=VROW)[:, :, VW * h:VW * h + HD],
                            v_sb[:, :, HD * h:HD * h + HD])
                    for h in range(H):
                        nc.sync.dma_start(
                            v_dst.rearrange("(tt p c) -> p tt c", p=128,
                                            c=VROW)
                            [:, :, VW * h + HD:VW * h + HD + 1],
                            ones_v[:, :, h:h + 1])

                    nc.gpsimd.collective_compute(
                        "AllGather", AX.bypass,
                        ins=[kv_own.opt()],
                        outs=[kv_g.opt()],
                        replica_groups=[[0, 1, 2, 3], [4, 5, 6, 7]],
                    )

                    wq_sb = wpool.tile([128, DT, D], BF16, tag="w")
                    nc.sync.dma_start(
                        wq_sb[:],
                        p["wqT"][:].rearrange("(a p) o -> p a o", p=128))
                    q_sb = kvq.tile([128, DT, T], BF16, name="q_sb")
                    for o in range(DT):
                        ps = psC.tile([128, T], F32, tag="pp")
                        for i in range(DT):
                            nc.tensor.matmul(
                                ps[:], wq_sb[:, i, o * 128:(o + 1) * 128],
                                nx[:, i, :], start=(i == 0),
                                stop=(i == DT - 1))
                        nc.vector.tensor_copy(q_sb[:, o, :], ps[:])

                # prefetch wo while attention runs
                wo_sb = wpool.tile([128, DT, D], BF16, tag="w")
                nc.sync.dma_start(
                    wo_sb[:],
                    p["woT"][:].rearrange("(a p) o -> p a o", p=128))

                # ---- attention ----
                attn = kvq.tile([128, DT, T], BF16, name="attn_sb")
                with (
                    tc.tile_pool(name="kvg", bufs=1) as kvgp,
                    tc.tile_pool(name="att_sb", bufs=3) as asb,
                    tc.tile_pool(name="psS", bufs=2, space="PSUM") as psS,
                    tc.tile_pool(name="psAV", bufs=2, space="PSUM") as psAV,
                ):
                    kg = kvgp.tile([128, R, DT, T], BF16, name="kg")
                    vg = kvgp.tile([128, R, NT, VROW], BF16, name="vg")
                    for r in range(R):
                        nc.sync.dma_start(
                            kg[:, r, :, :],
                            kv_g[r, 0:K_ELEMS].rearrange(
                                "(a p t) -> p a t", p=128, t=T))
                        nc.sync.dma_start(
                            vg[:, r, :, :],
                            kv_g[r, K_ELEMS:KV_ELEMS].rearrange(
                                "(tt p c) -> p tt c", p=128, c=VROW))
                    for h in range(H):
                        pb = 64 * (h % 2)
                        a = h // 2
                        av = psAV.tile([VW, T], F32, tag="av")
                        for r in range(R):
                            for jt in range(NT):
                                q0 = jt * 128
                                ncols = T - q0
                                sc = psS.tile([128, T], F32, tag="sc")
                                nc.tensor.matmul(
                                    sc[:, 0:ncols],
                                    kg[pb:pb + HD, r, a, q0:q0 + 128],
                                    q_sb[pb:pb + HD, a, q0:T],
                                    start=True, stop=True)
                                nc.vector.tensor_tensor(
                                    sc[:, 0:128], sc[:, 0:128], cm[:, r, :],
                                    AX.add)
                                es = asb.tile([128, T], BF16, tag="es")
                                nc.scalar.activation(
                                    es[:, 0:ncols], sc[:, 0:ncols], EXP,
                                    bias=seqb[:, r, jt:jt + 1], scale=SCALE)
                                nc.tensor.matmul(
                                    av[:, q0:T],
                                    vg[:, r, jt, VW * h:VW * h + VW],
                                    es[:, 0:ncols],
                                    start=(r == 0 and jt == 0),
                                    stop=(r == R - 1 and jt == NT - 1),
                                    skip_group_check=True)
                        recip = asb.tile([1, T], F32, tag="recip", bufs=2)
                        nc.vector.reciprocal(recip[:], av[HD:VW, :])
                        bc = psS.tile([64, T], F32, tag="bc")
                        nc.tensor.matmul(bc[:], ones_row[0:1, 0:HD], recip[:],
                                         start=True, stop=True)
                        bcs = asb.tile([64, T], F32, tag="bcs", bufs=2)
                        nc.vector.tensor_copy(bcs[:], bc[:])
                        nc.vector.tensor_tensor(attn[pb:pb + HD, a, :],
                                                av[0:HD, :], bcs[:], AX.mult)

                # ---- out projection + residual (in-place into xt) ----
                with tc.tile_pool(name="psE", bufs=3, space="PSUM") as psE:
                    for o in range(DT):
                        ps = psE.tile([128, T], F32, tag="pp")
                        for i in range(DT):
                            nc.tensor.matmul(
                                ps[:], wo_sb[:, i, o * 128:(o + 1) * 128],
                                attn[:, i, :], start=(i == 0),
                                stop=(i == DT - 1))
                        nc.vector.tensor_tensor(xt[:, o, :], ps[:],
                                                xt[:, o, :], AX.add)

            # ---- LN2 (reuse nx), MLP ----
            _layernorm(tc, wk, xt, gb["g2"], gb["b2"], ones_col, ones_row, nx)

            with (
                tc.tile_pool(name="hpool", bufs=1) as hp,
                tc.tile_pool(name="w1p", bufs=1) as w1p,
                tc.tile_pool(name="psM", bufs=3, space="PSUM") as psM,
            ):
                hsb = hp.tile([128, FT, T], BF16, name="hsb")
                w1r = p["w1T"][:].rearrange("(a p) o -> p a o", p=128)
                for o in range(FT):
                    w1t = w1p.tile([128, DT, 128], BF16, tag="w1", bufs=4)
                    nc.sync.dma_start(w1t[:], w1r[:, :, o * 128:(o + 1) * 128])
                    ps = psM.tile([128, T], F32, tag="mm")
                    for i in range(DT):
                        nc.tensor.matmul(ps[:], w1t[:, i, :], nx[:, i, :],
                                         start=(i == 0), stop=(i == DT - 1))
                    nc.scalar.activation(hsb[:, o, :], ps[:], GELU,
                                         bias=bm1c[:, o:o + 1])

                w2r = p["w2T"][:].rearrange("(a p) o -> p a o", p=128)
                for o in range(DT):
                    w2t = w1p.tile([128, FT, 128], BF16, tag="w2", bufs=2)
                    nc.sync.dma_start(w2t[:], w2r[:, :, o * 128:(o + 1) * 128])
                    ps = psM.tile([128, T], F32, tag="mm")
                    for i in range(FT):
                        nc.tensor.matmul(ps[:], w2t[:, i, :], hsb[:, i, :],
                                         start=(i == 0), stop=(i == FT - 1))
                    tmp = wk.tile([128, T], F32, tag="otmp")
                    nc.vector.tensor_scalar(tmp[:], ps[:], bm2c[:, o:o + 1],
                                            None, AX.add)
                    nc.vector.tensor_tensor(xt[:, o, :], tmp[:], xt[:, o, :],
                                            AX.add)
                nc.sync.dma_start(
                    out_h[:].rearrange("(a p) t -> p a t", p=128), xt[:])

    nc.compile()
    return nc


def _tri_masks():
    i = np.arange(128)
    m_le = np.where(i[:, None] > i[None, :], NEG, 0.0).astype(np.float32)
    m_lt = np.where(i[:, None] >= i[None, :], NEG, 0.0).astype(np.float32)
    return m_le, m_lt


def _in_maps(inputs):
    x = np.asarray(inputs["x"], np.float32)
    sm = np.asarray(inputs["seq_mask"]).astype(np.uint8)
    shared = {
        "wqT": np.ascontiguousarray(np.asarray(inputs["wq"], np.float32).T).astype(NPBF16),
        "wkT": np.ascontiguousarray(np.asarray(inputs["wk"], np.float32).T).astype(NPBF16),
        "wvT": np.ascontiguousarray(np.asarray(inputs["wv"], np.float32).T).astype(NPBF16),
        "woT": np.ascontiguousarray(np.asarray(inputs["wo"], np.float32).T).astype(NPBF16),
        "w1T": np.ascontiguousarray(np.asarray(inputs["w_mlp1"], np.float32).T).astype(NPBF16),
        "w2T": np.ascontiguousarray(np.asarray(inputs["w_mlp2"], np.float32).T).astype(NPBF16),
        "g1": np.asarray(inputs["g1"], np.float32).reshape(1, D),
        "b1": np.asarray(inputs["b1"], np.float32).reshape(1, D),
        "g2": np.asarray(inputs["g2"], np.float32).reshape(1, D),
        "b2": np.asarray(inputs["b2"], np.float32).reshape(1, D),
        "bm1": np.asarray(inputs["b_mlp1"], np.float32),
        "bm2": np.asarray(inputs["b_mlp2"], np.float32),
    }
    m_le, m_lt = _tri_masks()
    maps = []
    for c in range(8):
        b, r = divmod(c, R)
        toks = np.arange(r, L, R)
        cmask = np.stack([m_le if rp <= r else m_lt for rp in range(R)])
        maps.append({
            "xT": np.ascontiguousarray(x[b, toks, :].T),
            "sm": np.ascontiguousarray(sm[b]),
            "cmask": np.ascontiguousarray(cmask),
            **shared,
        })
    return maps


def _run(inputs, **kw):
    if "nc" not in _CACHE:
        _CACHE["nc"] = _build()
    nc = _CACHE["nc"]
    res = run_bass_kernel_spmd(nc, _in_maps(inputs), core_ids=list(range(8)),
                               **kw)
    x = np.asarray(inputs["x"], np.float32)
    y = np.empty_like(x)
    for c in range(8):
        b, r = divmod(c, R)
        toks = np.arange(r, L, R)
        y[b, toks, :] = res.results[c]["out"].T
    return y, res


def kernel(**inputs):
    y, _ = _run(inputs)
    return y

